# revision 2
# baseline (speedup 1.0000x reference)
"""MinGRU (parallel log-space scan) Trainium2 Bass kernel.

Problem (hardcoded):
    x:    [B=8, S=4096, D=1024] f32
    W_hg: [D=1024, 2*D=2048]    f32
    out:  [B=8, S=4096, D=1024] f32

    hg = x @ W_hg ; hidden, gate = split(hg)
    h_t = (1-z_t) * h_{t-1} + z_t * g(hidden_t),  z = sigmoid(gate),
    g(v) = v + 0.5 if v >= 0 else sigmoid(v)  ==  max(v + 0.5, sigmoid(v))

Sharding: data-parallel over batch, one batch row per NeuronCore (8 cores),
W_hg replicated.

Per-core pipeline (chunks of C=512 timesteps):
  DMA x chunk -> PE transpose (x^T, channels-on-partitions for the matmul)
  -> fp32r matmuls hg^T = W^T x^T accumulated over D in PSUM
  -> ACT sigmoids + DVE scalar_tensor_tensor fusions
  -> DVE tensor_tensor_scan (the minGRU linear recurrence along seq)
  -> PE transpose back -> DMA out.
"""

import os

import numpy as np

import concourse.bacc as bacc
import concourse.tile as tile
from concourse import mybir

B, S, D = 8, 4096, 1024
N_CORES = 8
P = 128  # partitions
C = 512  # seq chunk
N_CHUNKS = S // C  # 8
N_SSUB = C // P  # 4 s-subtiles per chunk
N_DT = D // P  # 8 d-tiles
N_KT = D // P  # 8 output channel tiles (hidden dim = D)

F32 = mybir.dt.float32
# fp32r: full-rate PE (1 cyc/row at N>=256) with TF32-class precision.
# Set MINGRU_MM_F32=1 to fall back to exact fp32 matmuls (4x slower PE).
MM_DT = F32 if os.environ.get("MINGRU_MM_F32") else mybir.dt.float32r

_COMPILED = {}


def _build():
    nc = bacc.Bacc(
        "TRN2", target_bir_lowering=False, debug=False, num_devices=N_CORES
    )
    x_d = nc.dram_tensor("x", [S, D], MM_DT, kind="ExternalInput").ap()
    w_d = nc.dram_tensor("w", [D, 2 * D], MM_DT, kind="ExternalInput").ap()
    id_r_d = nc.dram_tensor("ident_r", [P, P], MM_DT, kind="ExternalInput").ap()
    id_f_d = nc.dram_tensor("ident_f", [P, P], F32, kind="ExternalInput").ap()
    out_d = nc.dram_tensor("out", [S, D], F32, kind="ExternalOutput").ap()

    AL = mybir.AluOpType
    SIG = mybir.ActivationFunctionType.Sigmoid

    with tile.TileContext(nc) as tc:
        with (
            tc.tile_pool(name="consts", bufs=1) as consts,
            tc.tile_pool(name="wpool", bufs=1) as wpool,
            tc.tile_pool(name="xnat", bufs=2) as xnat_pool,
            tc.tile_pool(name="xtp", bufs=2) as xt_pool,
            tc.tile_pool(name="pw", bufs=2) as pw_pool,
            tc.tile_pool(name="hp", bufs=2) as h_pool,
            tc.tile_pool(name="psxt", bufs=2, space="PSUM") as psum_xt,
            tc.tile_pool(name="pshg", bufs=4, space="PSUM") as psum_hg,
            tc.tile_pool(name="psout", bufs=2, space="PSUM") as psum_out,
        ):
            ident_r = consts.tile([P, P], MM_DT, tag="identr")
            nc.sync.dma_start(ident_r[:], id_r_d[:])
            ident_f = consts.tile([P, P], F32, tag="identf")
            nc.sync.dma_start(ident_f[:], id_f_d[:])

            w_sb = []
            for j in range(N_DT):
                wt = wpool.tile([P, 2 * D], MM_DT, tag=f"w{j}")
                nc.sync.dma_start(wt[:], w_d[j * P : (j + 1) * P, :])
                w_sb.append(wt)

            prev_h = [None] * N_KT
            for sc in range(N_CHUNKS):
                s0 = sc * C
                # ---- load x chunk [C, D] as 4 natural [128, 1024] tiles
                xns = []
                for i in range(N_SSUB):
                    xn = xnat_pool.tile([P, D], MM_DT, tag=f"xn{i}")
                    r = s0 + i * P
                    nc.sync.dma_start(xn[:], x_d[r : r + P, :])
                    xns.append(xn)
                # ---- transpose to x^T tiles [128d, C]
                xts = []
                for j in range(N_DT):
                    pxt = psum_xt.tile([P, C], MM_DT, tag="pxt")
                    for i in range(N_SSUB):
                        nc.tensor.transpose(
                            pxt[:, i * P : (i + 1) * P],
                            xns[i][:, j * P : (j + 1) * P],
                            ident_r[:],
                        )
                    xt = xt_pool.tile([P, C], MM_DT, tag=f"xt{j}")
                    nc.scalar.copy(xt[:], pxt[:])
                    xts.append(xt)
                # ---- per channel-tile k: matmuls + pointwise + scan
                hs = []
                for k in range(N_KT):
                    ph = psum_hg.tile([P, C], F32, tag="ph")  # hidden
                    for j in range(N_DT):
                        nc.tensor.matmul(
                            ph[:],
                            w_sb[j][:, k * P : (k + 1) * P],
                            xts[j][:],
                            start=(j == 0),
                            stop=(j == N_DT - 1),
                        )
                    pg = psum_hg.tile([P, C], F32, tag="ph")  # gate
                    for j in range(N_DT):
                        nc.tensor.matmul(
                            pg[:],
                            w_sb[j][:, D + k * P : D + (k + 1) * P],
                            xts[j][:],
                            start=(j == 0),
                            stop=(j == N_DT - 1),
                        )
                    # a = sigmoid(-gate) = 1 - z
                    a_t = pw_pool.tile([P, C], F32, tag="a")
                    nc.scalar.activation(a_t[:], pg[:], SIG, scale=-1.0)
                    # sigh = sigmoid(hidden)
                    sigh = pw_pool.tile([P, C], F32, tag="sigh")
                    nc.scalar.activation(sigh[:], ph[:], SIG)
                    # g(hidden) = max(hidden + 0.5, sigmoid(hidden))
                    gh = pw_pool.tile([P, C], F32, tag="gh")
                    nc.vector.scalar_tensor_tensor(
                        gh[:], ph[:], 0.5, sigh[:], op0=AL.add, op1=AL.max
                    )
                    # bneg = (a - 1) * g = -(z * g)
                    bneg = pw_pool.tile([P, C], F32, tag="bneg")
                    nc.vector.scalar_tensor_tensor(
                        bneg[:], a_t[:], 1.0, gh[:], op0=AL.subtract, op1=AL.mult
                    )
                    # h_t = a_t * h_{t-1} - bneg_t  (linear recurrence)
                    h = h_pool.tile([P, C], F32, tag=f"h{k}")
                    init = 0.0 if prev_h[k] is None else prev_h[k][:, C - 1 : C]
                    nc.vector.tensor_tensor_scan(
                        h[:], a_t[:], bneg[:], init, op0=AL.mult, op1=AL.subtract
                    )
                    prev_h[k] = h
                    hs.append(h)
                # ---- transpose h back to [s, ch] and store
                for i in range(N_SSUB):
                    for half in range(2):
                        po = psum_out.tile([P, C], F32, tag="po")
                        for q in range(4):
                            k = half * 4 + q
                            nc.tensor.transpose(
                                po[:, q * P : (q + 1) * P],
                                hs[k][:, i * P : (i + 1) * P],
                                ident_f[:],
                            )
                        osb = pw_pool.tile([P, C], F32, tag="osb")
                        nc.scalar.copy(osb[:], po[:])
                        r = s0 + i * P
                        nc.sync.dma_start(
                            out_d[r : r + P, half * C : (half + 1) * C], osb[:]
                        )
    nc.compile()
    return nc


def _get_nc():
    key = str(MM_DT)
    if key not in _COMPILED:
        _COMPILED[key] = _build()
    return _COMPILED[key]


def kernel(x: np.ndarray, W_hg: np.ndarray) -> np.ndarray:
    from concourse.bass_utils import run_bass_kernel_spmd

    assert x.shape == (B, S, D) and W_hg.shape == (D, 2 * D)
    nc = _get_nc()
    ident = np.eye(P, dtype=np.float32)
    x = np.ascontiguousarray(x, dtype=np.float32)
    w = np.ascontiguousarray(W_hg, dtype=np.float32)
    in_maps = [
        {"x": x[b], "w": w, "ident_r": ident, "ident_f": ident}
        for b in range(N_CORES)
    ]
    res = run_bass_kernel_spmd(nc, in_maps, list(range(N_CORES)))
    out = np.stack([res.results[b]["out"] for b in range(N_CORES)], axis=0)
    return out.astype(np.float32)


# revision 11
# speedup vs baseline: 1.0421x; 1.0421x over previous
"""MinGRU (parallel log-space scan) Trainium2 Bass kernel.

Problem (hardcoded):
    x:    [B=8, S=4096, D=1024] f32
    W_hg: [D=1024, 2*D=2048]    f32
    out:  [B=8, S=4096, D=1024] f32

    hg = x @ W_hg ; hidden, gate = split(hg)
    h_t = (1-z_t) * h_{t-1} + z_t * g(hidden_t),  z = sigmoid(gate),
    g(v) = v + 0.5 if v >= 0 else sigmoid(v)  ==  max(v + 0.5, sigmoid(v))

Sharding: data-parallel over batch, one batch row per NeuronCore (8 cores),
W_hg replicated.

Per-core pipeline (chunks of C=512 timesteps):
  DMA x chunk -> PE transpose (x^T, channels-on-partitions for the matmul)
  -> fp32r matmuls hg^T = W^T x^T accumulated over D in PSUM
  -> ACT sigmoids + DVE scalar_tensor_tensor fusions
  -> DVE tensor_tensor_scan (the minGRU linear recurrence along seq)
  -> PE transpose back -> DMA out.
"""

import os

import numpy as np

import concourse.bacc as bacc
import concourse.tile as tile
from concourse import mybir

B, S, D = 8, 4096, 1024
N_CORES = 8
P = 128  # partitions
C = 512  # seq chunk
N_CHUNKS = S // C  # 8
N_SSUB = C // P  # 4 s-subtiles per chunk
N_DT = D // P  # 8 d-tiles
N_KT = D // P  # 8 output channel tiles (hidden dim = D)

F32 = mybir.dt.float32
# fp32r: full-rate PE (1 cyc/row at N>=256) with TF32-class precision.
# Set MINGRU_MM_F32=1 to fall back to exact fp32 matmuls (4x slower PE).
MM_DT = F32 if os.environ.get("MINGRU_MM_F32") else mybir.dt.float32r

_COMPILED = {}


def _build():
    nc = bacc.Bacc(
        "TRN2", target_bir_lowering=False, debug=False, num_devices=N_CORES
    )
    x_d = nc.dram_tensor("x", [S, D], MM_DT, kind="ExternalInput").ap()
    w_d = nc.dram_tensor("w", [D, 2 * D], MM_DT, kind="ExternalInput").ap()
    ID_R_DT = MM_DT
    id_r_d = nc.dram_tensor("ident_r", [P, P], ID_R_DT, kind="ExternalInput").ap()
    id_f_d = nc.dram_tensor("ident_f", [P, P], F32, kind="ExternalInput").ap()
    out_d = nc.dram_tensor("out", [S, D], F32, kind="ExternalOutput").ap()

    AL = mybir.AluOpType
    SIG = mybir.ActivationFunctionType.Sigmoid

    with tile.TileContext(nc) as tc:
        with (
            tc.tile_pool(name="consts", bufs=1) as consts,
            tc.tile_pool(name="wpool", bufs=1) as wpool,
            tc.tile_pool(name="xnat", bufs=2) as xnat_pool,
            tc.tile_pool(name="xtp", bufs=2) as xt_pool,
            tc.tile_pool(name="pw", bufs=2) as pw_pool,
            tc.tile_pool(name="hp", bufs=2) as h_pool,
            tc.tile_pool(name="psxt", bufs=2, space="PSUM") as psum_xt,
            tc.tile_pool(name="pshg", bufs=4, space="PSUM") as psum_hg,
            tc.tile_pool(name="psout", bufs=2, space="PSUM") as psum_out,
        ):
            ident_r = consts.tile([P, P], ID_R_DT, tag="identr")
            nc.sync.dma_start(ident_r[:], id_r_d[:])
            ident_f = consts.tile([P, P], F32, tag="identf")
            nc.sync.dma_start(ident_f[:], id_f_d[:])

            w_sb = []
            for j in range(N_DT):
                wt = wpool.tile([P, 2 * D], MM_DT, tag=f"w{j}")
                nc.sync.dma_start(wt[:], w_d[j * P : (j + 1) * P, :])
                w_sb.append(wt)

            prev_h = [None] * N_KT

            def emit_outputs(hs, s0):
                # transpose h back to [s, ch] and store
                for i in range(N_SSUB):
                    for half in range(2):
                        po = psum_out.tile([P, C], F32, tag="po")
                        for q in range(4):
                            k = half * 4 + q
                            nc.tensor.transpose(
                                po[:, q * P : (q + 1) * P],
                                hs[k][:, i * P : (i + 1) * P],
                                ident_f[:],
                            )
                        osb = pw_pool.tile([P, C], F32, tag="osb")
                        nc.scalar.copy(osb[:], po[:])
                        r = s0 + i * P
                        nc.sync.dma_start(
                            out_d[r : r + P, half * C : (half + 1) * C], osb[:]
                        )

            pending = None  # (hs, s0) of the previous chunk
            for sc in range(N_CHUNKS):
                s0 = sc * C
                # ---- load x chunk [C, D] as 4 natural [128, 1024] tiles
                xns = []
                for i in range(N_SSUB):
                    xn = xnat_pool.tile([P, D], MM_DT, tag=f"xn{i}")
                    r = s0 + i * P
                    nc.sync.dma_start(xn[:], x_d[r : r + P, :])
                    xns.append(xn)
                # ---- transpose to x^T tiles [128d, C]
                xts = []
                for j in range(N_DT):
                    pxt = psum_xt.tile([P, C], MM_DT, tag="pxt")
                    for i in range(N_SSUB):
                        nc.tensor.transpose(
                            pxt[:, i * P : (i + 1) * P],
                            xns[i][:, j * P : (j + 1) * P],
                            ident_r[:],
                        )
                    xt = xt_pool.tile([P, C], MM_DT, tag=f"xt{j}")
                    nc.scalar.copy(xt[:], pxt[:])
                    xts.append(xt)
                # ---- per channel-tile k: matmuls + pointwise + scan
                hs = []
                for k in range(N_KT):
                    ph = psum_hg.tile([P, C], F32, tag="ph")  # hidden
                    for j in range(N_DT):
                        nc.tensor.matmul(
                            ph[:],
                            w_sb[j][:, k * P : (k + 1) * P],
                            xts[j][:],
                            start=(j == 0),
                            stop=(j == N_DT - 1),
                        )
                    pg = psum_hg.tile([P, C], F32, tag="ph")  # gate
                    for j in range(N_DT):
                        nc.tensor.matmul(
                            pg[:],
                            w_sb[j][:, D + k * P : D + (k + 1) * P],
                            xts[j][:],
                            start=(j == 0),
                            stop=(j == N_DT - 1),
                        )
                    # a = sigmoid(-gate) = 1 - z
                    a_t = pw_pool.tile([P, C], F32, tag="a")
                    nc.scalar.activation(a_t[:], pg[:], SIG, scale=-1.0)
                    # sigh = sigmoid(hidden)
                    sigh = pw_pool.tile([P, C], F32, tag="sigh")
                    nc.scalar.activation(sigh[:], ph[:], SIG)
                    # g(hidden) = max(hidden + 0.5, sigmoid(hidden))
                    gh = pw_pool.tile([P, C], F32, tag="gh")
                    nc.vector.scalar_tensor_tensor(
                        gh[:], ph[:], 0.5, sigh[:], op0=AL.add, op1=AL.max
                    )
                    # bneg = (a - 1) * g = -(z * g)
                    bneg = pw_pool.tile([P, C], F32, tag="bneg")
                    nc.vector.scalar_tensor_tensor(
                        bneg[:], a_t[:], 1.0, gh[:], op0=AL.subtract, op1=AL.mult
                    )
                    # h_t = a_t * h_{t-1} - bneg_t  (linear recurrence)
                    h = h_pool.tile([P, C], F32, tag=f"h{k}")
                    init = 0.0 if prev_h[k] is None else prev_h[k][:, C - 1 : C]
                    nc.vector.tensor_tensor_scan(
                        h[:], a_t[:], bneg[:], init, op0=AL.mult, op1=AL.subtract
                    )
                    prev_h[k] = h
                    hs.append(h)
                # software pipelining: the previous chunk's output transposes
                # land in the PE stream here, filling the gap while this
                # chunk's pointwise/scan tail completes on ACT/DVE.
                if pending is not None:
                    emit_outputs(*pending)
                pending = (hs, s0)
            emit_outputs(*pending)
    nc.compile()
    return nc


def _get_nc():
    key = str(MM_DT)
    if key not in _COMPILED:
        _COMPILED[key] = _build()
    return _COMPILED[key]


def kernel(x: np.ndarray, W_hg: np.ndarray) -> np.ndarray:
    from concourse.bass_utils import run_bass_kernel_spmd

    assert x.shape == (B, S, D) and W_hg.shape == (D, 2 * D)
    nc = _get_nc()
    ident = np.eye(P, dtype=np.float32)
    ident_r = ident
    x = np.ascontiguousarray(x, dtype=np.float32)
    w = np.ascontiguousarray(W_hg, dtype=np.float32)
    in_maps = [
        {"x": x[b], "w": w, "ident_r": ident_r, "ident_f": ident}
        for b in range(N_CORES)
    ]
    res = run_bass_kernel_spmd(nc, in_maps, list(range(N_CORES)))
    out = np.stack([res.results[b]["out"] for b in range(N_CORES)], axis=0)
    return out.astype(np.float32)
